# revision 24
# baseline (speedup 1.0000x reference)
"""Differential attention kernel for Trainium2, 8-core SPMD.

Problem: B=2, S=2048, D=1024, 16 heads x 64 head-dim differential attention
(two softmaxes, combined with a scalar lambda), with input/output projections.

Sharding: data-parallel over batch (2 groups of 4 cores) x tensor-parallel
over heads (4 heads per core). Each core computes q/k/v projections for its
4 heads, both attention softmaxes, and a partial output projection
(its heads' rows of Wo). Host sums the 4 partial outputs per batch, adds bo.

v4 notes (on top of the v2 transposed-PV pipeline):
  - Scores are computed transposed, sT[k, q] in [128, 2*QB] PSUM tiles
    (branch 1 cols 0:QB, branch 2 cols QB:2QB). Most chunks exp on ACT
    straight out of PSUM (mask folded into the per-partition bias); 2 chunks
    per window exp on DVE via a Schraudolph bit-trick: one tensor_scalar
    (mult+add) writing int32 = the f32 bit pattern of exp(x); the PV matmuls
    read the high 16 bits of each int32 as bf16 through a strided AP view.
    This takes ACT off the critical path (ACT was pacing the whole kernel).
  - PV runs "transposed": et chunks are the stationary operand and v_aug the
    moving operand ([128k, 65]), accumulated into [128, 512] PSUM bank tiles
    (4 q-blocks at 128-col offsets), memset-zeroed, start=False.
  - Softmax sums arrive via the appended ones-column (col 64 of each slot).
  - stg ([q, hd] per head) is rebuilt to [hd, q] via PE transpose, then the
    out-projection contracts K=128 head-pairs (2 matmuls per [128d, 512q]).
  - DMA generation and transfer are serial resources: the tiny constants
    (mask/Schraudolph bias/identity bits) ship as ONE packed f32 tensor
    first, then wq/wk head-0 columns, then hs j0 in halves, then the rest.
    Identity matmuls warm the PE p-state ramp until real operands land.
  - Deferred work drains in fine grain (q-proj and k-proj are separate
    entries) so no multi-us PE burst ever blocks the score->exp stream.
  - PSUM evictions alternate DVE/ACT for transposes and out-proj tiles, and
    each j's 8 out-proj tiles land in one staging tile shipped by a single
    DMA, shrinking the tail.
All matmuls run in bf16 with fp32 PSUM accumulation; output partials ship as
fp16 and are reduced across cores in fp32 on the host.
"""

import sys

sys.path.insert(0, "/opt/trn_rl_repo")

from contextlib import ExitStack

import ml_dtypes
import numpy as np

import concourse.bacc as bacc
import concourse.tile as tile
from concourse import mybir
from concourse.bass_utils import run_bass_kernel_spmd

B, S, D = 2, 2048, 1024
NH, HD = 16, 64
NCORES = 8
HPC = 4              # heads per core
QB = 512             # q block (free dim of score matmuls)
NJ = S // QB         # 4
KC = 128             # k chunk (partition dim of transposed scores)
NKC = S // KC        # 16
NDI = D // 128       # 8 contraction chunks for projections
VA = HD + 1          # v columns per head incl. ones column

BF16 = mybir.dt.bfloat16
F16 = mybir.dt.float16
F32 = mybir.dt.float32
I32 = mybir.dt.int32
npbf16 = ml_dtypes.bfloat16

# Schraudolph exp: exp(x) ~= bitcast_f32(int32(A*x + B)). A maps x into the
# exponent field; B = 127*2^23 - C (C centers the mantissa-linearization
# error, ~+-3% max) + 2^15 (centers bf16 truncation of the high half).
SCH_A = float(2.0 ** 23 / np.log(2.0))          # 12102203.16
SCH_B = float(127 * 2 ** 23 - 360916 + 32768)   # 1065025068
SCALE = float(HD) ** -0.5                        # 1/8

# Which score chunks of each (h, j) window run on DVE instead of ACT.
DVE_CHUNKS = (4, 9, 14)
WARM = 26            # PE p-state warm-up matmuls

# Module-level cache: the Bass module depends only on shapes and lambda.
_BUILD_CACHE = {}
TRACE = False
LAST_RESULTS = None


def _build(lam: float, with_bias: bool = True, repeat: int = 1):
    nc = bacc.Bacc(None, target_bir_lowering=False)

    hst_d = nc.dram_tensor("hst", [D, S], BF16, kind="ExternalInput")
    wq_d = nc.dram_tensor("wq", [D, 2 * HPC * HD], BF16, kind="ExternalInput")
    wk_d = nc.dram_tensor("wk", [D, 2 * HPC * HD], BF16, kind="ExternalInput")
    wv_d = nc.dram_tensor("wv", [D, HPC * HD], BF16, kind="ExternalInput")
    wo_d = nc.dram_tensor("wo", [HPC * HD, D], BF16, kind="ExternalInput")
    bq_d = nc.dram_tensor("bq", [1, 2 * HPC * HD], BF16, kind="ExternalInput")
    bk_d = nc.dram_tensor("bk", [1, 2 * HPC * HD], BF16, kind="ExternalInput")
    bv_d = nc.dram_tensor("bv", [1, HPC * HD], BF16, kind="ExternalInput")
    # packed constants: cols 0:16 mask bias, 16:32 Schraudolph bias,
    # 32:96 identity matrix bits (64 f32 = 128 bf16 per row).
    cst_d = nc.dram_tensor("constf", [128, 2 * NKC + 64], F32, kind="ExternalInput")
    out_d = nc.dram_tensor("outT", [D, S], F16, kind="ExternalOutput")

    WQW = 2 * HPC * HD  # column stride per chunk in wqt/wkt
    WVW = HPC * HD

    with tile.TileContext(nc) as tc, ExitStack() as ctx:
        const = ctx.enter_context(tc.tile_pool(name="const", bufs=1))
        wpool = ctx.enter_context(tc.tile_pool(name="wpool", bufs=1))
        hpool = ctx.enter_context(tc.tile_pool(name="hpool", bufs=1))
        qkpool = ctx.enter_context(tc.tile_pool(name="qkpool", bufs=1))
        vpool = ctx.enter_context(tc.tile_pool(name="vpool", bufs=1))
        epool = ctx.enter_context(tc.tile_pool(name="epool", bufs=22))
        e32pool = ctx.enter_context(tc.tile_pool(name="e32pool", bufs=3))
        rpool = ctx.enter_context(tc.tile_pool(name="rpool", bufs=3))
        tpool = ctx.enter_context(tc.tile_pool(name="tpool", bufs=4))
        slabp = ctx.enter_context(tc.tile_pool(name="slabp", bufs=1))
        sgpool = ctx.enter_context(tc.tile_pool(name="sgpool", bufs=1))
        opool = ctx.enter_context(tc.tile_pool(name="opool", bufs=1))
        ps_sc = ctx.enter_context(tc.tile_pool(name="ps_sc", bufs=2, space="PSUM"))
        ps_pv = ctx.enter_context(tc.tile_pool(name="ps_pv", bufs=2, space="PSUM"))
        ps_tr = ctx.enter_context(tc.tile_pool(name="ps_tr", bufs=2, space="PSUM"))

        # ---- DMA order matters: generation (~625ns each) and the transfers
        # themselves are serial resources in HW DGE. Critical chain first.
        constf = const.tile([128, 2 * NKC + 64], F32, tag="constf")
        nc.sync.dma_start(out=constf[:], in_=cst_d[:])
        maskt = constf[:, 0:NKC]
        dmaskt = constf[:, NKC:2 * NKC]
        ident = constf[:, 2 * NKC:2 * NKC + 64].bitcast(BF16)

        # Separate tiles for the head-0 weight columns and the first/second
        # halves of hs j0, so the first projection has NO dependency on the
        # bulk transfers (tile-granular DMA deps would otherwise serialize).
        WQR = WQW - 128  # columns for heads 1..3
        wq0t = wpool.tile([128, NDI * 128], BF16, tag="wq0", name="wq0t")
        wk0t = wpool.tile([128, NDI * 128], BF16, tag="wk0", name="wk0t")
        wqt = wpool.tile([128, NDI * WQR], BF16, tag="wq", name="wqt")
        wkt = wpool.tile([128, NDI * WQR], BF16, tag="wk", name="wkt")

        def load_wqk_cols(t, dram, lo, hi, eng):
            # head columns [lo:hi) of every 128-row chunk of [D, WQW] DRAM.
            w = hi - lo
            dstv = t[:].rearrange("p (c w) -> p c w", w=w)
            srcv = dram.rearrange("(c p) w -> p c w", p=128)
            eng.dma_start(out=dstv[:], in_=srcv[:, :, lo:hi])

        load_wqk_cols(wq0t, wq_d, 0, 128, nc.sync)
        load_wqk_cols(wk0t, wk_d, 0, 128, nc.sync)

        hs0a = hpool.tile([128, 4 * QB], BF16, tag="hs0a", name="hs0a")
        hs0b = hpool.tile([128, 4 * QB], BF16, tag="hs0b", name="hs0b")
        hs0s = hst_d[:, 0:QB].rearrange("(c p) w -> p c w", p=128)
        nc.sync.dma_start(out=hs0a[:].rearrange("p (c w) -> p c w", w=QB),
                          in_=hs0s[:, 0:4])
        nc.sync.dma_start(out=hs0b[:].rearrange("p (c w) -> p c w", w=QB),
                          in_=hs0s[:, 4:8])

        def load_folded(pool, dram, width, tag, name, eng):
            # [rows, width] DRAM -> [128, (rows/128) * width] SBUF, chunk-major
            nch = dram.shape[0] // 128
            t = pool.tile([128, nch * width], BF16, tag=tag, name=name)
            eng.dma_start(
                out=t[:].rearrange("p (c w) -> p c w", w=width),
                in_=dram.rearrange("(c p) w -> p c w", p=128),
            )
            return t

        # One queue, strict priority order: the generator round-robins
        # across queues, so a second queue would let bulk transfers jump
        # ahead of the critical chain on the serial DMA resource.
        hstj = [None] * NJ
        wvt = load_folded(wpool, wv_d, WVW, "wv", "wvt", nc.sync)
        load_wqk_cols(wkt, wk_d, 128, WQW, nc.sync)
        load_wqk_cols(wqt, wq_d, 128, WQW, nc.sync)
        hstj[1] = load_folded(hpool, hst_d[:, QB:2 * QB], QB, "hs1", "hs1", nc.sync)
        hstj[2] = load_folded(hpool, hst_d[:, 2 * QB:3 * QB], QB, "hs2", "hs2",
                              nc.sync)
        hstj[3] = load_folded(hpool, hst_d[:, 3 * QB:4 * QB], QB, "hs3", "hs3",
                              nc.sync)
        wot = load_folded(wpool, wo_d, D, "wo", "wot", nc.sync)

        def hsc(j, c):
            # [128, QB] hidden-state chunk c of block j.
            if j == 0:
                t = hs0a if c < 4 else hs0b
                return t[:, (c % 4) * QB:(c % 4 + 1) * QB]
            return hstj[j][:, c * QB:(c + 1) * QB]
        if with_bias:
            bqt = const.tile([1, 2 * HPC * HD], BF16, tag="bq")
            nc.gpsimd.dma_start(out=bqt[:], in_=bq_d[:])
            bkt = const.tile([1, 2 * HPC * HD], BF16, tag="bk")
            nc.gpsimd.dma_start(out=bkt[:], in_=bk_d[:])
            bvt = const.tile([1, HPC * HD], BF16, tag="bv")
            nc.gpsimd.dma_start(out=bvt[:], in_=bv_d[:])
        else:
            bqt = bkt = bvt = None
        ones = const.tile([1, S], BF16, tag="ones")
        nc.gpsimd.memset(ones[:], 1.0)

        # PE p-state warm-up: identity matmuls keep the tensor engine
        # continuously busy from ~2.5us so the 3us ramp to full clock
        # completes before the first projection's operands land.
        for i in range(WARM):
            wps = ps_tr.tile([128, 128], F32, tag="tr", name=f"warm{i}")
            nc.tensor.matmul(wps[:], lhsT=ident, rhs=ident, start=True, stop=True)

        def qk_proj_mms(ps, which, h, j, c0, c1):
            # which: 0 = q, 1 = k. Head 0 reads the dedicated early tiles.
            if h == 0:
                wt, w, lo = ((wq0t, wk0t)[which], 128, 0)
            else:
                wt, w, lo = ((wqt, wkt)[which], WQR, (h - 1) * 128)
            for c in range(c0, c1):
                nc.tensor.matmul(
                    ps[:],
                    lhsT=wt[:, c * w + lo:c * w + lo + 128],
                    rhs=hsc(j, c),
                    start=(c == 0),
                    stop=(not with_bias and c == NDI - 1),
                )

        def qk_proj_finish(ps, which, dst, h, j):
            if with_bias:
                bt = (bqt, bkt)[which]
                nc.tensor.matmul(
                    ps[:],
                    lhsT=bt[0:1, h * 128:h * 128 + 128],
                    rhs=ones[0:1, j * QB:(j + 1) * QB],
                    start=False,
                    stop=True,
                )
            nc.vector.tensor_copy(dst[:], ps[:])

        def emit_qk_proj_one(h, j, which):
            # one of q/k: 8 chunk matmuls + bias + eviction, a ~1.8us entry.
            dsts = (qt, kt)[which]
            ps = ps_tr.tile([128, QB], F32, tag="tr", name=f"pj{h}_{j}_{which}")
            qk_proj_mms(ps, which, h, j, 0, NDI)
            qk_proj_finish(ps, which, dsts[h][j], h, j)

        def emit_v_proj_chunk(sc):
            # v[s, 4*64] for s-chunk sc, scattered into v_aug (65-wide head
            # blocks, ones column preset by memset).
            ps = ps_tr.tile([128, HPC * HD], F32, tag="tr")
            for c in range(NDI):
                nc.tensor.matmul(
                    ps[:],
                    lhsT=hsc(sc // 4, c)[:, (sc % 4) * 128:(sc % 4 + 1) * 128],
                    rhs=wvt[:, c * WVW:(c + 1) * WVW],
                    start=(c == 0),
                    stop=(not with_bias and c == NDI - 1),
                )
            if with_bias:
                nc.tensor.matmul(
                    ps[:],
                    lhsT=ones[0:1, 0:128],
                    rhs=bvt[0:1, :],
                    start=False,
                    stop=True,
                )
            src = ps[:].rearrange("p (h x) -> p h x", x=HD)
            dst = va[sc][:].rearrange("p (h y) -> p h y", y=VA)[:, :, 0:HD]
            nc.vector.tensor_copy(dst, src)

        NQB = QB // 128  # 4 q sub-blocks per j

        # Pipelined emission: deferred work (PV quads, norm, transposes,
        # out-proj groups, next-head projections) sits in a FIFO and drains
        # between score-matmul halves of the CURRENT stream. Quads are 4
        # matmuls (= PE wait-queue depth), so a quad whose exp has not
        # retired parks in the wait queue while later scores execute around
        # it; window tails drain inside the next window, so the exp engines
        # never see a boundary gap. RESERVE keeps ~2.5 chunks of backlog so
        # drained quads' exps have retired.
        pending = []
        p_head = [0]
        RESERVE = 7

        def drain(nmax, reserve=None):
            r = RESERVE if reserve is None else reserve
            done = 0
            while done < nmax and len(pending) - p_head[0] > r:
                pending[p_head[0]]()
                p_head[0] += 1
                done += 1

        def flush_pending():
            while p_head[0] < len(pending):
                pending[p_head[0]]()
                p_head[0] += 1

        def emit_attn(j, h, per_chunk=None, dve_chunks=(), last=False):
            # No memset: chunk 0 / q-block 0 opens each branch's PSUM bank
            # with start=True (clears the bank's has_written bits, so the
            # sibling q-block slots overwrite on first touch and accumulate
            # after) - the quads drain in (c, qb) order, which makes this
            # safe and removes the boundary norm->memset DVE serialization.
            pvs = []
            for br in range(2):
                pv = ps_pv.tile([128, QB], F32, tag="pv", name=f"pv{j}_{h}_{br}")
                pvs.append(pv)

            def make_quad(c, br, esl):
                def quad():
                    for qb in range(NQB):
                        nc.tensor.matmul(
                            pvs[br][:, qb * 128:qb * 128 + VA],
                            lhsT=esl(br, qb),
                            rhs=va[c][:, h * VA:(h + 1) * VA],
                            start=(c == 0 and qb == 0),
                            stop=(c == NKC - 1),
                            skip_group_check=True,
                        )
                return quad

            for c in range(NKC):
                sp = ps_sc.tile([128, 2 * QB], F32, tag="sp")
                kj, kcol = divmod(c * KC, QB)
                nc.tensor.matmul(
                    sp[:, 0:QB],
                    lhsT=kt[h][kj][0:64, kcol:kcol + KC],
                    rhs=qt[h][j][0:64, :],
                    start=True,
                    stop=True,
                )
                bk = len(pending) - p_head[0]
                if c == 0:
                    pass  # window's first score pair goes out unimpeded
                elif last and c >= 10:
                    drain(1, 2)
                else:
                    drain(1 if bk < 20 else 2)
                nc.tensor.matmul(
                    sp[:, QB:2 * QB],
                    lhsT=kt[h][kj][64:128, kcol:kcol + KC],
                    rhs=qt[h][j][64:128, :],
                    start=True,
                    stop=True,
                )
                if c in dve_chunks:
                    # Schraudolph exp on DVE: int32 out = f32 bits of exp.
                    et32 = e32pool.tile([128, 2 * QB], I32, tag="e32")
                    nc.vector.tensor_scalar(
                        et32[:],
                        sp[:],
                        SCH_A * SCALE,
                        dmaskt[:, c:c + 1],
                        mybir.AluOpType.mult,
                        mybir.AluOpType.add,
                    )
                    hiv = (et32[:].bitcast(BF16)
                           .rearrange("p (c two) -> p c two", two=2)[:, :, 1:2])

                    def esl(br, qb, hiv=hiv):
                        return hiv[:, br * QB + qb * 128:br * QB + (qb + 1) * 128]
                else:
                    et = epool.tile([128, 2 * QB], BF16, tag="et")
                    nc.scalar.activation(
                        et[:],
                        sp[:],
                        mybir.ActivationFunctionType.Exp,
                        bias=maskt[:, c:c + 1],
                        scale=SCALE,
                    )

                    def esl(br, qb, et=et):
                        return et[:, br * QB + qb * 128:br * QB + (qb + 1) * 128]

                bk = len(pending) - p_head[0]
                if c == 0:
                    pass
                elif last and c >= 10:
                    drain(3, 2)
                else:
                    drain(1 if bk < 20 else 2)
                if per_chunk is not None:
                    per_chunk(c)
                pending.append(make_quad(c, 0, esl))
                pending.append(make_quad(c, 1, esl))
            pending.append(lambda: emit_norm(j, h, pvs))

        def emit_norm(j, h, pvs):
            pv1, pv2 = pvs
            # normalization: out = pv1/r1 - lam * pv2/r2, with sums at col
            # 64 of each 128-col slot; per-partition scalars via
            # tensor_scalar / scalar_tensor_tensor.
            rz = rpool.tile([128, 3 * NQB], F32, tag="rz")
            sums1 = pv1[:].rearrange("p (q c) -> p q c", c=128)[:, :, VA - 1:VA]
            sums2 = pv2[:].rearrange("p (q c) -> p q c", c=128)[:, :, VA - 1:VA]
            nc.vector.reciprocal(out=rz[:, 0:NQB], in_=sums1)
            nc.vector.reciprocal(out=rz[:, NQB:2 * NQB], in_=sums2)
            nc.vector.tensor_scalar_mul(
                rz[:, 2 * NQB:3 * NQB], rz[:, NQB:2 * NQB], float(-lam)
            )
            hp, hh = divmod(h, 2)
            for qb in range(NQB):
                t1 = tpool.tile([128, HD], F32, tag="t1")
                nc.vector.tensor_scalar_mul(
                    t1[:], pv1[:, qb * 128:qb * 128 + HD], rz[:, qb:qb + 1]
                )
                nc.vector.scalar_tensor_tensor(
                    out=slab[j][qb][hp][:, hh * HD:(hh + 1) * HD],
                    in0=pv2[:, qb * 128:qb * 128 + HD],
                    scalar=rz[:, 2 * NQB + qb:2 * NQB + qb + 1],
                    in1=t1[:],
                    op0=mybir.AluOpType.mult,
                    op1=mybir.AluOpType.add,
                )

        def emit_transpose(j, hp):
            # stg [q, hd-pair] -> [hd-pair, q] via PE transpose; evictions
            # alternate DVE/ACT so neither engine paces the chain alone.
            for qb in range(NQB):
                tp = ps_tr.tile([128, 128], BF16, tag="tr", name=f"tp{j}_{hp}_{qb}")
                nc.tensor.matmul(
                    tp[:],
                    lhsT=slab[j][qb][hp][:],
                    rhs=ident,
                    is_transpose=True,
                    start=True,
                    stop=True,
                )
                dst = stg_pair[j][hp][:, qb * 128:(qb + 1) * 128]
                # ACT helps only in the post-exp tail; mid-kernel its strict
                # FIFO would park copies ahead of the exp stream.
                if j == NJ - 1 and hp == 1 and qb % 2 == 1:
                    nc.scalar.activation(
                        dst, tp[:], mybir.ActivationFunctionType.Copy)
                else:
                    nc.vector.tensor_copy(dst, tp[:])

        def emit_outproj_d(j, d):
            # partial out-projection: outT[do, qblock] = sum_hp wo_hp.T @
            # stg_hp, staged into the per-j [128, NDI*QB] f16 tile (one DMA
            # per j). Evictions alternate DVE/ACT.
            ps = ps_tr.tile([128, QB], F32, tag="tr", name=f"op{j}_{d}")
            for hp in range(HPC // 2):
                nc.tensor.matmul(
                    ps[:],
                    lhsT=wot[:, hp * D + d * 128:hp * D + (d + 1) * 128],
                    rhs=stg_pair[j][hp][:],
                    start=(hp == 0),
                    stop=(hp == HPC // 2 - 1),
                )
            dst = obig[j % 2][:, d * QB:(d + 1) * QB]
            if j == NJ - 1 and d % 2 == 1:
                nc.scalar.activation(dst, ps[:], mybir.ActivationFunctionType.Copy)
            else:
                nc.vector.tensor_copy(dst, ps[:])
            # ship pairs of d-tiles so the final transfer overlaps the
            # remaining evictions instead of trailing them all.
            if d % 2 == 1:
                nc.sync.dma_start(
                    out=out_d[(d - 1) * 128:(d + 1) * 128, j * QB:(j + 1) * QB]
                        .rearrange("(c p) w -> p c w", p=128),
                    in_=obig[j % 2][:, (d - 1) * QB:(d + 1) * QB]
                        .rearrange("p (c w) -> p c w", w=QB),
                )

        # ---- emission order: heads outer so the exp stream has no gaps;
        # v-proj and head h+1's projections fill PE slack during attention;
        # transposes run after each head-pair, out-projection per j as soon
        # as the last head's stage lands (only j=NJ-1 is a true tail).
        for _rep in range(repeat):
            qt = [[qkpool.tile([128, QB], BF16, tag=f"qt{h}_{j}", name=f"qt{h}_{j}")
                   for j in range(NJ)] for h in range(HPC)]
            kt = [[qkpool.tile([128, QB], BF16, tag=f"kt{h}_{j}", name=f"kt{h}_{j}")
                   for j in range(NJ)] for h in range(HPC)]
            va = [vpool.tile([128, HPC * VA], BF16, tag=f"va{c}", name=f"va{c}")
                  for c in range(NKC)]
            slab = [[[slabp.tile([128, 2 * HD], BF16, tag=f"sl{j}_{qb}_{hp}",
                                 name=f"sl{j}_{qb}_{hp}")
                      for hp in range(HPC // 2)] for qb in range(NQB)]
                    for j in range(NJ)]
            stg_pair = [[sgpool.tile([128, QB], BF16, tag=f"sg{j}_{hp}",
                                     name=f"sg{j}_{hp}")
                         for hp in range(HPC // 2)] for j in range(NJ)]
            obig = [opool.tile([128, NDI * QB], F16, tag=f"ob{i}", name=f"ob{i}")
                    for i in range(2)]
            for c in range(NKC):
                nc.gpsimd.memset(va[c][:], 1.0)

            # proj(0, 0) runs before the first scores, staged around the two
            # hs-j0 DMA halves: q/k chunks 0-3 can start once the first half
            # lands. The remaining head-0 projections are injected just-in-
            # time inside window (0, 0) (scores chunk c reads kt[0][c // 4],
            # so proj(0, kj) must precede chunk 4*kj), as are the v-proj
            # chunks (pvt(c) needs va[c], emitted with LAG >= 3 after chunk
            # c). Head h+1's projections then spread across head h's row.
            pjq = ps_tr.tile([128, QB], F32, tag="tr", name="pjq0")
            pjk = ps_tr.tile([128, QB], F32, tag="tr", name="pjk0")
            qk_proj_mms(pjq, 0, 0, 0, 0, 4)
            qk_proj_mms(pjk, 1, 0, 0, 0, 4)
            qk_proj_mms(pjq, 0, 0, 0, 4, NDI)
            qk_proj_finish(pjq, 0, qt[0][0], 0, 0)
            qk_proj_mms(pjk, 1, 0, 0, 4, NDI)
            qk_proj_finish(pjk, 1, kt[0][0], 0, 0)

            def first_window_jit(c):
                pending.append(lambda c=c: emit_v_proj_chunk(c))
                if c in (2, 6, 10):
                    emit_qk_proj_one(0, c // 4 + 1, 0)
                elif c in (3, 7, 11):
                    emit_qk_proj_one(0, c // 4 + 1, 1)

            for h in range(HPC):
                for j in range(NJ):
                    first = (h, j) == (0, 0)
                    last = (h, j) == (HPC - 1, NJ - 1)
                    emit_attn(j, h,
                              per_chunk=first_window_jit if first else None,
                              dve_chunks=() if first else DVE_CHUNKS,
                              last=last)
                    if h % 2 == 1:
                        pending.append(
                            lambda j=j, hp=h // 2: emit_transpose(j, hp))
                    if h == HPC - 1:
                        for d in range(NDI):
                            pending.append(lambda j=j, d=d: emit_outproj_d(j, d))
                    if h < HPC - 1:
                        pending.append(
                            lambda h=h, j=j: emit_qk_proj_one(h + 1, j, 0))
                        pending.append(
                            lambda h=h, j=j: emit_qk_proj_one(h + 1, j, 1))
            flush_pending()

    nc.compile()
    return nc


def _prep_inputs(hidden_states, attention_mask, Wq, bq, Wk, bk, Wv, bv, Wo):
    """Build the 8 per-core input maps (host-side shard + transpose + cast)."""
    in_maps = []
    hsT = [np.ascontiguousarray(hidden_states[b].T).astype(npbf16) for b in range(B)]
    identf = np.ascontiguousarray(np.eye(128, dtype=npbf16)).view(np.float32)
    constf = []
    for b in range(B):
        maskcol = ((1.0 - attention_mask[b]) * -10000.0).astype(np.float32)
        maskcol = maskcol.reshape(NKC, KC).T
        # Schraudolph per-partition bias: A * clamp(mask, -80) + B. (Masked
        # positions land at tiny positive exp values instead of i32 overflow.)
        dmaskc = (SCH_A * np.maximum(maskcol, -80.0) + SCH_B).astype(np.float32)
        constf.append(np.ascontiguousarray(
            np.concatenate([maskcol, dmaskc, identf], axis=1)))
    for core in range(NCORES):
        b = core // (NCORES // B)
        hb = (core % (NCORES // B)) * HPC
        heads = range(hb, hb + HPC)
        qk_idx = np.concatenate(
            [np.r_[h * HD:(h + 1) * HD, D + h * HD:D + (h + 1) * HD] for h in heads]
        )
        v_idx = np.r_[hb * HD:(hb + HPC) * HD]
        in_maps.append(
            {
                "hst": hsT[b],
                "wq": np.ascontiguousarray(Wq[:, qk_idx]).astype(npbf16),
                "wk": np.ascontiguousarray(Wk[:, qk_idx]).astype(npbf16),
                "wv": np.ascontiguousarray(Wv[:, v_idx]).astype(npbf16),
                "wo": np.ascontiguousarray(Wo[v_idx, :]).astype(npbf16),
                "bq": bq[qk_idx].reshape(1, -1).astype(npbf16),
                "bk": bk[qk_idx].reshape(1, -1).astype(npbf16),
                "bv": bv[v_idx].reshape(1, -1).astype(npbf16),
                "constf": constf[b],
            }
        )
    return in_maps


def kernel(
    hidden_states,
    attention_mask,
    Wq,
    bq,
    Wk,
    bk,
    Wv,
    bv,
    Wo,
    bo,
    lq1,
    lk1,
    lq2,
    lk2,
):
    global LAST_RESULTS
    args = [hidden_states, attention_mask, Wq, bq, Wk, bk, Wv, bv, Wo, bo]
    hidden_states, attention_mask, Wq, bq, Wk, bk, Wv, bv, Wo, bo = (
        np.asarray(a, dtype=np.float32) for a in args
    )
    lq1, lk1, lq2, lk2 = (np.asarray(a, dtype=np.float64) for a in (lq1, lk1, lq2, lk2))
    lam = float(np.exp(lq1 @ lk1) - np.exp(lq2 @ lk2) + 0.8)

    with_bias = not (
        np.all(bq == 0) and np.all(bk == 0) and np.all(bv == 0)
    )
    key = (round(lam, 9), with_bias)
    if key not in _BUILD_CACHE:
        _BUILD_CACHE.clear()
        _BUILD_CACHE[key] = _build(lam, with_bias)
    nc = _BUILD_CACHE[key]

    in_maps = _prep_inputs(hidden_states, attention_mask, Wq, bq, Wk, bk, Wv, bv, Wo)
    res = run_bass_kernel_spmd(nc, in_maps, core_ids=list(range(NCORES)), trace=TRACE)
    LAST_RESULTS = res

    out = np.empty((B, S, D), dtype=np.float32)
    gpb = NCORES // B
    for b in range(B):
        acc = res.results[b * gpb]["outT"].astype(np.float32)
        for g in range(1, gpb):
            acc = acc + res.results[b * gpb + g]["outT"]
        out[b] = acc.T + bo[None, :]
    return out
